# revision 21
# baseline (speedup 1.0000x reference)
"""4-layer GAT (GATConv x4 + log_softmax) on 8 Trainium2 NeuronCores.

Strategy (node/row sharding + edge-parallel segment softmax):
  - Core c owns node rows [c*NPC, (c+1)*NPC).
  - Per layer l:
    Phase A: h = x @ Waug  row-sharded on each core's own nodes, fp16
             inputs with fp32 PSUM accumulation.  Waug = [W | W@a_src |
             W@a_dst], so the two extra columns of the matmul output are
             the per-node scores ss/sd for free.  Rows [h | ss | sd |
             0pad] (fp16, padded to a 128-elem multiple so gather rows
             are 256B-aligned) written to local DRAM staging.
    Phase B: the per-core staging rows are AllGathered (RDH) into TWO
             shared tables split by local row range: T1 = rows [0, B1)
             of every core, T2 = rows [B1, NPC).  AG1 fires as soon as
             the first B1T row-tiles of phase A are done (mid tile
             loop), AG2 at the end; phase A of layer l+1 is fused into
             phase C of layer l tile-by-tile, so both collectives
             overlap edge aggregation and matmul work instead of
             serializing between layers.
    Phase C: edge aggregation for the core's own dst nodes. Edges are
             bucketed by (core, dst-tile, src-half) on the host and
             padded to C1 chunks (sources in T1) + C2 chunks (sources
             in T2) of 128 edges.  Source rows [h|ss] are fetched with
             batched dma_gather instructions (segments of <=SEG chunks;
             idx blocks replicated across the 8 Q7 cores' 16-partition
             groups).  T1-sourced chunks are processed first, so work
             can start before AG2 lands.
             DVE work is batched per TILE over [128, CT, 128] broadcast
             access patterns:
               - one-hot S01[p, j, q] = (dl[p,j] == q), built ONCE at
                 layer 0 and cached in SBUF (the graph is layer-invariant)
               - sde = rowsum(S01 * sd_bcast)  (per-edge dst score)
               - e = ss_gathered + sde;  w = exp(leakyrelu(e)) on the
                 scalar engine (Lrelu + Exp)
               - S = S01 * w  (scaled one-hot)
             Then fp16 PE matmuls accumulate per chunk
             psum[dst, :] += S_j^T @ [h rows | 1] giving the weighted
             message sum and (ones rhs column) the softmax denominator.
             out = num/den (+relu via ACT with per-row scale, or
             log_softmax for the last layer).
  - Softmax max-subtraction is skipped: logits are O(+-10) for this
    model family so exp() cannot overflow in fp32; the result is
    mathematically identical to the max-subtracted form.
"""

import numpy as np

import concourse.bass as bass
import concourse.bacc as bacc
import concourse.mybir as mybir
import concourse.tile as tile
from concourse import bass_utils
from concourse.masks import make_identity

NCORES = 8
P = 128
SEG = 6                      # gather chunks per dma_gather segment
NEG_SLOPE = 0.2
EPS = 1e-16
F32 = mybir.dt.float32
F16 = mybir.dt.float16
I16 = mybir.dt.int16
AF = mybir.ActivationFunctionType


def _pad_to(n, mult):
    return ((n + mult - 1) // mult) * mult


class Cfg:
    def __init__(self, N, dims, C1, C2):
        assert N % NCORES == 0
        self.N = N
        self.dims = dims                      # [(din, dout), ...]
        self.C1 = C1                          # T1-sourced chunks per dst tile
        self.C2 = C2                          # T2-sourced chunks per dst tile
        self.CT = C1 + C2
        self.NPC = N // NCORES                # nodes per core
        self.NT = (self.NPC + P - 1) // P     # dst tiles per core
        self.nlast = self.NPC - (self.NT - 1) * P
        # src-half split: first B1T tiles' rows go to T1, rest to T2
        self.B1T = self.NT   # single shared table (split AG not profitable)
        self.B1 = min(self.B1T * P, self.NPC)
        self.B2 = self.NPC - self.B1
        # padded row width of the augmented table: >= dout+2, 128-elem
        # aligned so fp16 gather rows are a 256B multiple (dma_gather req)
        self.pads = [_pad_to(dout + 2, P) for _, dout in dims]
        self.xtpad = _pad_to(self.NPC, P)     # padded node column count


def make_cfg(N, dims, edge_index):
    """Compute C1/C2 from the edge data (self-loops appended)."""
    npc = N // NCORES
    nt = (npc + P - 1) // P
    b1t = nt
    b1 = min(b1t * P, npc)
    src = np.concatenate([np.asarray(edge_index[0]),
                          np.arange(N, dtype=np.int64)]).astype(np.int64)
    dst = np.concatenate([np.asarray(edge_index[1]),
                          np.arange(N, dtype=np.int64)]).astype(np.int64)
    c_of = dst // npc
    r = dst - c_of * npc
    t_of = r // P
    half = ((src % npc) >= b1).astype(np.int64)
    key = (c_of * nt + t_of) * 2 + half
    counts = np.bincount(key, minlength=NCORES * nt * 2).reshape(-1, 2)
    C1 = int(np.ceil(counts[:, 0].max() / P))
    C2 = int(np.ceil(counts[:, 1].max() / P))
    return Cfg(N, dims, C1, C2)


def col_splits(width):
    """Split [0,width) into PSUM-bank-sized matmul column regions (<=512)."""
    out = []
    c = 0
    while c < width:
        out.append((c, min(c + 512, width)))
        c += 512
    return out


def segs(C):
    return [(s0, min(s0 + SEG, C)) for s0 in range(0, C, SEG)]


def build_program(cfg: Cfg, xt_bufs=None):
    nl = len(cfg.dims)
    nc = bacc.Bacc("TRN2", num_devices=NCORES)

    # ---- external inputs ----
    din0 = cfg.dims[0][0]
    xT_d = nc.dram_tensor("xT", [din0, cfg.xtpad], F16, kind="ExternalInput")
    W_d = [
        nc.dram_tensor(f"W{l}", [cfg.dims[l][0], cfg.dims[l][1] + 2], F16,
                       kind="ExternalInput")
        for l in range(nl)
    ]
    idx_d = nc.dram_tensor("idx", [cfg.NT, P, cfg.CT * 8], I16,
                           kind="ExternalInput")
    dl_d = nc.dram_tensor("dl", [cfg.NT, P, cfg.CT], F16, kind="ExternalInput")
    iota_d = nc.dram_tensor("iota", [P, P], F16, kind="ExternalInput")
    dlast = cfg.dims[-1][1]
    out_d = nc.dram_tensor("out", [cfg.NPC, dlast], F32, kind="ExternalOutput")

    maxpad = max(cfg.pads)
    maxdout = max(dout for _, dout in cfg.dims)
    maxkc = max(d // P for d, _ in cfg.dims)
    if xt_bufs is None:
        xt_bufs = min(12, 2 * maxkc)
    NT, B1T = cfg.NT, cfg.B1T

    with tile.TileContext(nc) as tc:
        with (
            tc.tile_pool(name="xt", bufs=xt_bufs) as xt_pool,
            tc.tile_pool(name="w", bufs=2 * maxkc + 2) as w_pool,
            tc.tile_pool(name="g", bufs=4) as g_pool,
            tc.tile_pool(name="stg", bufs=3) as stg_pool,
            tc.tile_pool(name="s01c", bufs=NT) as s01c_pool,
            tc.tile_pool(name="dlc", bufs=NT) as dlc_pool,
            tc.tile_pool(name="scr", bufs=2) as scr_pool,
            tc.tile_pool(name="sseg", bufs=4) as sseg_pool,
            tc.tile_pool(name="small", bufs=4) as small_pool,
            tc.tile_pool(name="sdcol", bufs=2 * NT + 2) as sdcol_pool,
            tc.tile_pool(name="consts", bufs=1) as const_pool,
            tc.tile_pool(name="acc", bufs=2, space="PSUM") as acc_pool,
            tc.tile_pool(name="aux", bufs=2, space="PSUM") as aux_pool,
            tc.tile_pool(name="tpose", bufs=2, space="PSUM") as tp_pool,
            tc.tile_pool(name="dram", bufs=1, space="DRAM") as dram_pool,
        ):
            ident = const_pool.tile([P, P], F16, tag="ident")
            make_identity(nc, ident[:])
            iota_sb = const_pool.tile([P, P], F16, tag="iota")
            nc.sync.dma_start(out=iota_sb[:], in_=iota_d[:, :])
            ones_sb = const_pool.tile([P, 16], F16, tag="ones")
            nc.vector.memset(ones_sb[:], 1.0)

            # per-tile caches living across all layers
            idx_tiles = []
            dl_tiles = []
            for m in range(NT):
                it = const_pool.tile([P, cfg.CT * 8], I16, tag=f"idxc{m}")
                nc.sync.dma_start(out=it[:], in_=idx_d[m, :, :])
                idx_tiles.append(it)
                dt_ = dlc_pool.tile([P, cfg.CT], F16, tag="dlc", name=f"dlc{m}")
                nc.sync.dma_start(out=dt_[:], in_=dl_d[m, :, :])
                dl_tiles.append(dt_)
            s01_cache = [None] * NT

            # lhsT chunks of the current layer input ([P, xtpad] each)
            xt_cur = []
            for k in range(cfg.dims[0][0] // P):
                t = xt_pool.tile([P, cfg.xtpad], F16, tag="xt")
                nc.sync.dma_start(out=t[:], in_=xT_d[k * P:(k + 1) * P, :])
                xt_cur.append(t)

            def load_w(l):
                dout = cfg.dims[l][1]
                tiles = []
                for k in range(cfg.dims[l][0] // P):
                    t = w_pool.tile([P, maxdout + 2], F16, tag="w")
                    nc.sync.dma_start(out=t[:, 0:dout + 2],
                                      in_=W_d[l][k * P:(k + 1) * P, :])
                    tiles.append(t)
                return tiles

            def alloc_tables(l):
                pad = cfg.pads[l]
                a1 = dram_pool.tile([cfg.B1, pad], F16, tag=f"agin1_{l}")
                t1 = dram_pool.tile([NCORES * cfg.B1, pad], F16,
                                    tag=f"t1_{l}", addr_space="Shared")
                if cfg.B2:
                    a2 = dram_pool.tile([cfg.B2, pad], F16, tag=f"agin2_{l}")
                    t2 = dram_pool.tile([NCORES * cfg.B2, pad], F16,
                                        tag=f"t2_{l}", addr_space="Shared")
                else:
                    a2 = t2 = None
                return a1, a2, t1, t2

            def start_ag(tabs, part):
                a1, a2, t1, t2 = tabs
                ins = a1 if part == 1 else a2
                outs = t1 if part == 1 else t2
                nc.gpsimd.collective_compute(
                    "AllGather",
                    mybir.AluOpType.bypass,
                    replica_groups=[list(range(NCORES))],
                    ins=[ins[:, :].opt()],
                    outs=[outs[:, :].opt()],
                )

            def phase_a_tile(l, m, xt_src, w_tiles, tabs, sd_cols):
                din, dout = cfg.dims[l]
                pad = cfg.pads[l]
                kc = din // P
                rows = P if m < NT - 1 else cfg.nlast
                a1, a2, _, _ = tabs
                ph = acc_pool.tile([P, maxdout], F32, space="PSUM", tag="acc")
                px = aux_pool.tile([P, 16], F32, space="PSUM", tag="aux")
                for k in range(kc):
                    for (a, b) in col_splits(dout):
                        nc.tensor.matmul(
                            out=ph[:, a:b],
                            lhsT=xt_src[k][:, m * P:(m + 1) * P],
                            rhs=w_tiles[k][:, a:b],
                            start=(k == 0),
                            stop=(k == kc - 1),
                        )
                    nc.tensor.matmul(
                        out=px[:, 0:2],
                        lhsT=xt_src[k][:, m * P:(m + 1) * P],
                        rhs=w_tiles[k][:, dout:dout + 2],
                        start=(k == 0),
                        stop=(k == kc - 1),
                    )
                stg = stg_pool.tile([P, maxpad], F16, tag="stg")
                nc.scalar.activation(out=stg[:, 0:dout], in_=ph[:, 0:dout],
                                     func=AF.Copy)
                nc.vector.tensor_copy(out=stg[:, dout:dout + 2], in_=px[:, 0:2])
                if pad > dout + 2:
                    nc.vector.memset(stg[:, dout + 2:pad], 0.0)
                sdc = sdcol_pool.tile([P, 1], F16, tag="sdc")
                nc.vector.tensor_copy(out=sdc[:], in_=stg[:, dout + 1:dout + 2])
                sd_cols.append(sdc)
                if m < B1T:
                    nc.sync.dma_start(
                        out=a1[m * P:m * P + rows, :],
                        in_=stg[0:rows, 0:pad])
                else:
                    r0 = (m - B1T) * P
                    nc.sync.dma_start(
                        out=a2[r0:r0 + rows, :],
                        in_=stg[0:rows, 0:pad])

            # ---- peeled phase A of layer 0 ----
            w_cur = load_w(0)
            tabs_cur = alloc_tables(0)
            sd_cur = []
            for m in range(NT):
                phase_a_tile(0, m, xt_cur, w_cur, tabs_cur, sd_cur)
                if m == B1T - 1:
                    start_ag(tabs_cur, 1)
            if cfg.B2:
                start_ag(tabs_cur, 2)

            for l in range(nl):
                din, dout = cfg.dims[l]
                pad = cfg.pads[l]
                last = l == nl - 1
                _, _, t1, t2 = tabs_cur

                if not last:
                    w_next = load_w(l + 1)
                    tabs_next = alloc_tables(l + 1)
                    sd_next = []
                    xt_next = [
                        xt_pool.tile([P, cfg.xtpad], F16, tag="xt",
                                     name=f"xtn{l}_{k}")
                        for k in range(dout // P)
                    ]

                # (half, seg_range, chunk_offset) list
                seg_list = [(t1, s0, s1, 0) for (s0, s1) in segs(cfg.C1)]
                if cfg.C2:
                    seg_list += [(t2, s0, s1, cfg.C1)
                                 for (s0, s1) in segs(cfg.C2)]

                # pre-compute per-tile dst-score vectors BEFORE the
                # gathers: this work depends only on phase-A outputs, so
                # DVE/PE fill the AllGather wait window with it
                sde_tiles = []
                for m in range(NT):
                    if l == 0:
                        sc = s01c_pool.tile([P, cfg.CT, P], F16, tag="s01c",
                                            name=f"s01c{m}")
                        dl3 = dl_tiles[m][:].unsqueeze(2).to_broadcast(
                            [P, cfg.CT, P])
                        iota3 = iota_sb[:].unsqueeze(1).to_broadcast(
                            [P, cfg.CT, P])
                        nc.vector.tensor_tensor(
                            out=sc[:], in0=dl3, in1=iota3,
                            op=mybir.AluOpType.is_equal)
                        s01_cache[m] = sc
                    ptp = tp_pool.tile([P, P], F16, space="PSUM", tag="tp")
                    nc.tensor.transpose(
                        out=ptp[:], in_=sd_cur[m][:, 0:1].to_broadcast([P, P]),
                        identity=ident[:],
                    )
                    sdbc = small_pool.tile([P, P], F16, tag="sdbc")
                    nc.vector.tensor_copy(out=sdbc[:], in_=ptp[:])
                    scr = scr_pool.tile([P, cfg.CT, P], F16, tag="scr")
                    nc.vector.tensor_mul(
                        out=scr[:], in0=s01_cache[m][:],
                        in1=sdbc[:].unsqueeze(1).to_broadcast([P, cfg.CT, P]))
                    sde_all = sdcol_pool.tile([P, cfg.CT], F32, tag="sde",
                                            name=f"sde{l}_{m}")
                    nc.vector.reduce_sum(out=sde_all[:], in_=scr[:],
                                         axis=mybir.AxisListType.X)
                    sde_tiles.append(sde_all)

                for m in range(NT):
                    rows = P if m < NT - 1 else cfg.nlast

                    # gather segments of source rows (flat tiles + rearrange
                    # keep the dest contiguous for every layer's pad)
                    g_segs = []
                    for (tab, s0, s1, off) in seg_list:
                        gt = g_pool.tile([P, SEG * maxpad], F16, tag="g")
                        gv = gt[:, 0:(s1 - s0) * pad].rearrange(
                            "p (a b) -> p a b", b=pad)
                        nc.gpsimd.dma_gather(
                            out_ap=gv,
                            in_ap=tab[:, :],
                            idxs_ap=idx_tiles[m][:, (off + s0) * 8:(off + s1) * 8],
                            num_idxs=(s1 - s0) * P,
                            num_idxs_reg=(s1 - s0) * P,
                            elem_size=pad,
                        )
                        g_segs.append(gv)

                    s01 = s01_cache[m]
                    sde_all = sde_tiles[m]

                    # per-segment: e -> w -> scaled one-hots -> matmuls,
                    # so PE starts on segment 0 while later segments gather
                    e_all = small_pool.tile([P, cfg.CT], F32, tag="eall")
                    w_all = small_pool.tile([P, cfg.CT], F32, tag="wall")
                    w16 = small_pool.tile([P, cfg.CT], F16, tag="w16")
                    po = acc_pool.tile([P, maxdout], F32, space="PSUM", tag="acc")
                    pd = aux_pool.tile([P, 16], F32, space="PSUM", tag="aux")
                    for si, (tab, s0, s1, off) in enumerate(seg_list):
                        ga, gb = off + s0, off + s1
                        nc.vector.tensor_add(
                            out=e_all[:, ga:gb],
                            in0=g_segs[si][:, :, dout:dout + 1].squeeze(2),
                            in1=sde_all[:, ga:gb],
                        )
                        nc.vector.tensor_scalar_mul(
                            out=w_all[:, ga:gb], in0=e_all[:, ga:gb],
                            scalar1=NEG_SLOPE)
                        nc.vector.tensor_tensor(
                            out=w_all[:, ga:gb], in0=w_all[:, ga:gb],
                            in1=e_all[:, ga:gb], op=mybir.AluOpType.max)
                        nc.scalar.activation(out=w16[:, ga:gb],
                                             in_=w_all[:, ga:gb], func=AF.Exp)
                        s_seg = sseg_pool.tile([P, SEG, P], F16, tag="sseg")
                        nc.vector.tensor_mul(
                            out=s_seg[:, 0:s1 - s0, :],
                            in0=s01[:, ga:gb, :],
                            in1=w16[:, ga:gb].unsqueeze(2).to_broadcast(
                                [P, s1 - s0, P]))
                        for j in range(s0, s1):
                            jg = off + j
                            for (a, b) in col_splits(dout):
                                nc.tensor.matmul(
                                    out=po[:, a:b], lhsT=s_seg[:, j - s0, :],
                                    rhs=g_segs[si][:, j - s0, a:b],
                                    start=(jg == 0), stop=(jg == cfg.CT - 1),
                                )
                            nc.tensor.matmul(
                                out=pd[:, 0:1], lhsT=s_seg[:, j - s0, :],
                                rhs=ones_sb[:, 0:1],
                                start=(jg == 0), stop=(jg == cfg.CT - 1),
                            )

                    # normalize: rec = 1/(den+eps)
                    dtmp = small_pool.tile([P, 1], F32, tag="dtmp")
                    nc.vector.tensor_scalar_add(
                        out=dtmp[:], in0=pd[:, 0:1], scalar1=EPS)
                    rec = small_pool.tile([P, 1], F32, tag="rec")
                    nc.vector.reciprocal(out=rec[:], in_=dtmp[:])

                    if not last:
                        relu_t = stg_pool.tile([P, maxpad], F16, tag="stg")
                        # relu(num * rec) on the scalar engine (bias is zero)
                        nc.scalar.activation(
                            out=relu_t[:, 0:dout], in_=po[:, 0:dout],
                            func=AF.Relu, scale=rec[:, 0:1])
                        for k in range(dout // P):
                            ptt = tp_pool.tile([P, P], F16, space="PSUM", tag="tp")
                            nc.tensor.transpose(
                                out=ptt[:], in_=relu_t[:, k * P:(k + 1) * P],
                                identity=ident[:],
                            )
                            nc.vector.tensor_copy(
                                out=xt_next[k][:, m * P:(m + 1) * P], in_=ptt[:])
                    else:
                        # log_softmax over features
                        t1o = small_pool.tile([P, dlast], F32, tag="t1")
                        nc.vector.tensor_scalar_mul(
                            out=t1o[:], in0=po[:, 0:dout], scalar1=rec[:, 0:1])
                        mx = small_pool.tile([P, 1], F32, tag="mx")
                        nc.vector.reduce_max(out=mx[:], in_=t1o[:],
                                             axis=mybir.AxisListType.X)
                        nc.vector.tensor_scalar_sub(
                            out=t1o[:], in0=t1o[:], scalar1=mx[:, 0:1])
                        ex = small_pool.tile([P, dlast], F32, tag="ex")
                        sm = small_pool.tile([P, 1], F32, tag="sm")
                        nc.scalar.activation(
                            out=ex[:], in_=t1o[:],
                            func=AF.Exp,
                            accum_out=sm[:])
                        lg = small_pool.tile([P, 1], F32, tag="lg")
                        nc.scalar.activation(
                            out=lg[:], in_=sm[:],
                            func=AF.Ln)
                        nc.vector.tensor_scalar_sub(
                            out=t1o[:], in0=t1o[:], scalar1=lg[:, 0:1])
                        nc.sync.dma_start(
                            out=out_d[m * P:m * P + rows, :],
                            in_=t1o[0:rows, :],
                        )

                if not last:
                    for m in range(NT):
                        phase_a_tile(l + 1, m, xt_next, w_next, tabs_next,
                                     sd_next)
                        if m == B1T - 1:
                            start_ag(tabs_next, 1)
                    if cfg.B2:
                        start_ag(tabs_next, 2)
                    xt_cur = xt_next
                    w_cur = w_next
                    tabs_cur = tabs_next
                    sd_cur = sd_next

    nc.compile()
    return nc


def prep_host(x, edge_index, Ws, a_srcs, a_dsts, cfg: Cfg):
    """Build per-core input maps."""
    N = cfg.N
    nl = len(cfg.dims)
    src = np.concatenate([np.asarray(edge_index[0]), np.arange(N, dtype=np.int64)])
    dst = np.concatenate([np.asarray(edge_index[1]), np.arange(N, dtype=np.int64)])
    src = src.astype(np.int64)
    dst = dst.astype(np.int64)

    c_of = dst // cfg.NPC
    r = dst - c_of * cfg.NPC
    t_of = r // P
    q = r - t_of * P
    # source side: owner core, local row, half, table-relative row
    sc = src // cfg.NPC
    sr = src - sc * cfg.NPC
    half = (sr >= cfg.B1).astype(np.int64)
    trow = np.where(half == 0, sc * cfg.B1 + sr,
                    sc * cfg.B2 + (sr - cfg.B1))

    key = (c_of * cfg.NT + t_of) * 2 + half
    order = np.argsort(key, kind="stable")
    counts = np.bincount(key, minlength=NCORES * cfg.NT * 2)
    cc = counts.reshape(-1, 2)
    need1 = int(np.ceil(cc[:, 0].max() / P))
    need2 = int(np.ceil(cc[:, 1].max() / P))
    assert need1 <= cfg.C1 and need2 <= cfg.C2, \
        f"need C1>={need1} C2>={need2}, got {cfg.C1},{cfg.C2}"

    idx_a = np.zeros((NCORES, cfg.NT, P, cfg.CT), dtype=np.int64)
    dl_a = np.full((NCORES, cfg.NT, P, cfg.CT), -1.0, dtype=np.float16)
    starts = np.zeros(NCORES * cfg.NT * 2 + 1, dtype=np.int64)
    np.cumsum(counts, out=starts[1:])
    for g in range(NCORES * cfg.NT * 2):
        seg = order[starts[g]:starts[g + 1]]
        if len(seg) == 0:
            continue
        ct, h = divmod(g, 2)
        c, t = divmod(ct, cfg.NT)
        k = np.arange(len(seg))
        jj = k // P + (0 if h == 0 else cfg.C1)
        pp = k % P
        idx_a[c, t, pp, jj] = trow[seg]
        dl_a[c, t, pp, jj] = q[seg]

    # dma_gather index layout: linear position k = chunk*128 + partition,
    # stored at [k % 16, k // 16], replicated across the 8 Q7 cores'
    # 16-partition groups
    idx16 = np.zeros((NCORES, cfg.NT, P, cfg.CT * 8), dtype=np.int16)
    for c in range(NCORES):
        for t in range(cfg.NT):
            lin = idx_a[c, t].T.reshape(-1)          # [CT*128], k=j*128+p
            blk = lin.reshape(cfg.CT * 8, 16).T.astype(np.int16)
            idx16[c, t] = np.tile(blk, (P // 16, 1))

    # augmented weights [W | W@a_src | W@a_dst], fp16
    Waug = []
    for l in range(nl):
        W = np.asarray(Ws[l], dtype=np.float32)
        was = W @ np.asarray(a_srcs[l], dtype=np.float32)
        wad = W @ np.asarray(a_dsts[l], dtype=np.float32)
        A = np.zeros((W.shape[0], W.shape[1] + 2), dtype=np.float16)
        A[:, :W.shape[1]] = W.astype(np.float16)
        A[:, W.shape[1]] = was.astype(np.float16)
        A[:, W.shape[1] + 1] = wad.astype(np.float16)
        Waug.append(A)

    iota = np.tile(np.arange(P, dtype=np.float16), (P, 1))

    x = np.asarray(x, dtype=np.float32)
    in_maps = []
    for c in range(NCORES):
        xs = x[c * cfg.NPC:(c + 1) * cfg.NPC]          # [NPC, din0]
        xT = np.zeros((cfg.dims[0][0], cfg.xtpad), dtype=np.float16)
        xT[:, :cfg.NPC] = xs.T.astype(np.float16)
        m = {
            "xT": np.ascontiguousarray(xT),
            "idx": np.ascontiguousarray(idx16[c]),
            "dl": np.ascontiguousarray(dl_a[c]),
            "iota": iota,
        }
        for l in range(nl):
            m[f"W{l}"] = Waug[l]
        in_maps.append(m)
    return in_maps


def run(x, edge_index, Ws, a_srcs, a_dsts, cfg: Cfg, trace=False):
    in_maps = prep_host(x, edge_index, Ws, a_srcs, a_dsts, cfg)
    nc = build_program(cfg)
    res = bass_utils.run_bass_kernel_spmd(
        nc, in_maps, core_ids=list(range(NCORES)), trace=trace)
    out = np.concatenate([res.results[c]["out"][:cfg.NPC] for c in range(NCORES)],
                         axis=0)
    return out, res


FULL_CFG_DIMS = [(256, 1024), (1024, 1024), (1024, 512), (512, 128)]


def _full_cfg(edge_index):
    return make_cfg(10000, FULL_CFG_DIMS, edge_index)


def kernel(x, edge_index, W1, as1, ad1, b1, W2, as2, ad2, b2,
           W3, as3, ad3, b3, W4, as4, ad4, b4):
    for b in (b1, b2, b3, b4):
        assert not np.any(np.asarray(b)), "nonzero bias not implemented"
    cfg = _full_cfg(edge_index)
    out, _ = run(
        x, edge_index,
        [W1, W2, W3, W4], [as1, as2, as3, as4], [ad1, ad2, ad3, ad4], cfg)
    return out.astype(np.float32)
